# revision 38
# baseline (speedup 1.0000x reference)
"""Trainium2 Bass kernel for FFNWithScales (SwiGLU MLP with low-rank dequant scales).

Reference computation (all fp32):
    gate_eff = gate_snapped * (gate_scale_A @ gate_scale_B)       # [8192, 2048]
    up_eff   = up_snapped   * (up_scale_A   @ up_scale_B)         # [8192, 2048]
    down_eff = down_snapped * (down_scale_A @ down_scale_B)       # [2048, 8192]
    h   = silu(gate_eff @ x) * (up_eff @ x)                       # [8192, 512]
    out = down_eff @ h                                            # [2048, 512]

Sharding (8 cores, tensor-parallel on d_ff): core c owns d_ff rows
[c*1024, (c+1)*1024) of gate/up (and the matching columns of down).
Each core computes a full-[2048, 512] partial of the down projection;
partials are summed on the host (the all-reduce step).

Kernel design (v2 — PE-bound, so everything serves the PE stream):
  - All tensors ship bf16 from the host (snapped weights included): the
    extra bf16 rounding of snapped costs ~1e-3 relative error against a
    2e-2 budget, and it halves HBM traffic so DMA (~17 MB @ ~300 GB/s)
    stays far under the PE streaming time.
  - Weights are pre-transposed on host so their contraction dim rides the
    partitions, and are DMA'd in full-row tiles (2-4 KB contiguous per
    partition line): gate/up as [128 d, 1024 f] per d-chunk, down as
    [128 f, 2048 d] per f-chunk.
  - The rank-32 scale products run 4-way row-packed on the PE
    (tile_position strips 0/32/64/96): one ~280 ns stream covers two
    d-chunks' worth of scale tiles. The DVE dequant-multiplies the bf16
    snapped tile by the fp32 psum scale tile, emitting the bf16 wr tile
    the main matmuls consume. Dequanted gate/up/down weights stay
    resident in SBUF so each of the 48 dequants serves two passes.
  - Main matmuls are kd-major in the first pass (matches the dequant
    feed rate) and fi-major-blocked afterwards, so a psum accumulator's
    epilogue (silu / up-multiply / output copy) always drains behind
    12+ matmuls on other banks — pass boundaries never stall the PE.
  - Six dummy warm-up matmuls on a memset tile run while the first DMAs
    land, so the PE HAM clock-gate reaches 2.4 GHz before real work.
  - Output partials store bf16 (host accumulates in fp32), with the
    final pass's stores split across both HWDGE rings for a short tail.
"""

import numpy as np
import ml_dtypes

import concourse.bass as bass
from concourse import bacc
import concourse.mybir as mybir
from concourse.tile import TileContext
from concourse.bass_utils import run_bass_kernel_spmd

P = 128
D = 2048        # d_model
FF = 8192       # d_ff (global)
S = 512         # sequence
R = 32          # rank
NCORES = 8
F = FF // NCORES          # 1024 local d_ff rows
KD = D // P               # 16 d_model chunks
KF = F // P               # 8 local d_ff chunks

f32 = mybir.dt.float32
bf16 = mybir.dt.bfloat16

_CACHE = {}


def _build():
    nc = bacc.Bacc()
    x = nc.declare_dram_parameter("x", [D, S], bf16, isOutput=False)
    # snapped weights, transposed, tiled so a dram slice is an SBUF tile
    gT = nc.declare_dram_parameter("gT", [KD, P, 2, 512], bf16, isOutput=False)
    uT = nc.declare_dram_parameter("uT", [KD, P, 2, 512], bf16, isOutput=False)
    dT = nc.declare_dram_parameter("dT", [KF, P, 4, 512], bf16, isOutput=False)
    # 4-way packed scale factors: B strips for chunk pairs (both fg copies),
    # A^T replicated on all four 32-row strips.
    gB4 = nc.declare_dram_parameter("gB4", [4 * R, KD // 2, P], bf16, isOutput=False)
    uB4 = nc.declare_dram_parameter("uB4", [4 * R, KD // 2, P], bf16, isOutput=False)
    dB4 = nc.declare_dram_parameter("dB4", [4 * R, KF // 2, P], bf16, isOutput=False)
    gAT4 = nc.declare_dram_parameter("gAT4", [4 * R, F], bf16, isOutput=False)
    uAT4 = nc.declare_dram_parameter("uAT4", [4 * R, F], bf16, isOutput=False)
    dAT4 = nc.declare_dram_parameter("dAT4", [4 * R, D], bf16, isOutput=False)
    # boot = [gB4 pair 0 | gAT4] in one transfer so scale-burst 0 can fire
    # ~1.7 us earlier than waiting for both full factor loads
    boot = nc.declare_dram_parameter("boot", [4 * R, P + F], bf16, isOutput=False)
    out = nc.declare_dram_parameter("out", [D, S], bf16, isOutput=True)

    silu = mybir.ActivationFunctionType.Silu

    with TileContext(nc) as tc:
        with (
            tc.tile_pool(name="const", bufs=1) as const,
            tc.tile_pool(name="wtg", bufs=6) as wtg,
            tc.tile_pool(name="wtd", bufs=8) as wtd,
            tc.tile_pool(name="gwr", bufs=1) as gwr,
            tc.tile_pool(name="uwr", bufs=1) as uwr,
            tc.tile_pool(name="dwr", bufs=1) as dwr,
            tc.tile_pool(name="hbuf", bufs=1) as hpool,
            tc.tile_pool(name="scb", bufs=3) as scbp,
            tc.tile_pool(name="obuf", bufs=3) as opool,
            tc.tile_pool(name="psacc", bufs=1, space="PSUM") as psacc,
            tc.tile_pool(name="pssc", bufs=2, space="PSUM") as pssc,
        ):
            # ---- constant loads (factors lead the rings, x0/x1 next) ----
            rounded = {}

            def load_const(nm, dram, eng):
                rt = const.tile(list(dram.shape), bf16, name=nm, tag=nm)
                eng.dma_start(rt, dram[:])
                rounded[nm] = rt

            load_const("boot", boot, nc.sync)

            x_sb = [None] * (KD // 2)

            def load_x_chunk(q, eng):
                xt = const.tile([P, 2, S], bf16, name=f"x{q}", tag=f"x{q}")
                eng.dma_start(
                    xt, x[q * 2 * P:(q + 1) * 2 * P, :].rearrange(
                        "(ko p) s -> p ko s", p=P))
                x_sb[q] = xt

            def xs(kd):
                return x_sb[kd // 2][:, kd % 2]

            # ---- PE warm-up: dummy matmuls cycling the ACC banks (NOT the
            # sc slots, which burst 0 needs as soon as `boot` lands).
            # Emitted in two chunks around the first dequant chain, because
            # the PE stream order is fixed at compile time: burst 0 must sit
            # only ~2 warmups deep so the chain starts the moment boot lands.
            junk = const.tile([P, 640], bf16, name="junk", tag="junk")
            nc.vector.memset(junk, 0.0)

            wu_ctr = [0]

            def warmup(n):
                for _ in range(n):
                    i = wu_ctr[0] % 4
                    wu_ctr[0] += 1
                    wps = psacc.tile([P, S], f32, name=f"acc{i}", tag=f"acc{i}")
                    nc.tensor.matmul(wps, junk[:, 0:128], junk[:, 128:640],
                                     start=True, stop=True)

            # ---- weight stream: 40 snapped-tile DMAs ----
            # jobs 0..15 gate kd, 16..31 up kd, 32..39 down kf
            wt_tiles = {}
            dma_parity = [0]

            def ring():
                dma_parity[0] ^= 1
                return nc.sync if dma_parity[0] else nc.scalar

            def emit_wt(j, eng=None):
                if j < 16:
                    t = wtg.tile([P, 2, 512], bf16, name="wt", tag="wt")
                    (eng or ring()).dma_start(t, gT[j])
                elif j < 32:
                    t = wtg.tile([P, 2, 512], bf16, name="wt", tag="wt")
                    (eng or ring()).dma_start(t, uT[j - 16])
                else:
                    t = wtd.tile([P, 4, 512], bf16, name="wtd", tag="wtd")
                    (eng or ring()).dma_start(t, dT[j - 32])
                wt_tiles[j] = t
                if j == 8:
                    # x3..x7 ride the idle gpsimd SWDGE ring once the
                    # startup-critical transfers are past the DMA engines
                    # (x3 first needed ~5 pair-blocks in)
                    for q in range(3, 8):
                        load_x_chunk(q, nc.gpsimd)
                if j == 7:
                    # up/down factors ride behind the first weight tiles
                    # (first needed ~30 us in, land ~18)
                    for nm, dram in (("uB4", uB4), ("uAT4", uAT4),
                                     ("dB4", dB4), ("dAT4", dAT4)):
                        load_const(nm, dram, ring())

            # ---- scale bursts + copies + dequants ----
            # dq jobs: 0..15 gate kd, 16..31 up kd, 32..47 down (h*8 + kf)
            # burst b covers dq jobs 2b, 2b+1. Each job is a 3-engine chain:
            # PE scale-matmul -> ACT psum->sbuf bf16 copy -> DVE bf16 2x
            # dequant multiply (all-16-bit keeps the DVE in its fast mode).
            sc_tiles = {}
            scb_tiles = {}
            dwr_tiles = {}
            dq_next = [0]
            wt_next = [0]

            def ensure_wt(upto):
                while wt_next[0] <= min(upto, 39):
                    emit_wt(wt_next[0])
                    wt_next[0] += 1

            def emit_burst(b):
                sc_a = pssc.tile([P, 2, S], f32, name="sc", tag="sc")
                sc_b = pssc.tile([P, 2, S], f32, name="sc", tag="sc")
                cols = [0, 512, 0, 512]
                if b == 0:         # gate pair 0 from the boot concat
                    bt = rounded["boot"]
                    Bsl = lambda i: bt[i * R:(i + 1) * R, 0:P]
                    Asl = lambda i, c: bt[i * R:(i + 1) * R, P + c:P + c + 512]
                elif b < 8:        # gate kd pair (2b, 2b+1)
                    Bm, Am = rounded["gB4"], rounded["gAT4"]
                    Bsl = lambda i: Bm[i * R:(i + 1) * R, b]
                    Asl = lambda i, c: Am[i * R:(i + 1) * R, c:c + 512]
                elif b < 16:       # up kd pair
                    Bm, Am = rounded["uB4"], rounded["uAT4"]
                    Bsl = lambda i: Bm[i * R:(i + 1) * R, b - 8]
                    Asl = lambda i, c: Am[i * R:(i + 1) * R, c:c + 512]
                else:              # down: m = b-16: h = m//4, kf pair j = m%4
                    m = b - 16
                    h = m // 4
                    Bm, Am, jd = rounded["dB4"], rounded["dAT4"], m % 4
                    Bsl = lambda i: Bm[i * R:(i + 1) * R, jd]
                    Asl = lambda i, c: Am[i * R:(i + 1) * R, c:c + 512]
                    cols = [h * 1024, h * 1024 + 512,
                            h * 1024, h * 1024 + 512]
                for i, dst in enumerate((sc_a[:, 0], sc_a[:, 1],
                                         sc_b[:, 0], sc_b[:, 1])):
                    nc.tensor.matmul(
                        dst, Bsl(i), Asl(i, cols[i]),
                        start=True, stop=True,
                        tile_position=(R * i, 0),
                    )
                sc_tiles[2 * b] = sc_a
                sc_tiles[2 * b + 1] = sc_b

            wr_of = {}

            def emit_dq(j):
                # three chain flavours, balanced across DVE/ACT/GPSIMD so no
                # single engine's backlog can stall the PE's sc-slot ring:
                #   j%4 in (0,2): DVE multiplies straight off the sc psum
                #   j%4 == 1:     ACT copies psum->sbuf, DVE multiplies (2x)
                #   j%4 == 3:     ACT copies psum->sbuf, GPSIMD multiplies
                # Chain flavours: gate (j<16) feeds pass 0 just-in-time AND
                # its pass ends right before the silus, so gate chains stay
                # entirely on the DVE (stable latency, ACT queue empty for
                # the silus). Up/down chains are prefetched with slack, so
                # they spread across ACT-copy + DVE/GPSIMD-multiply paths.
                # Chain flavours, balanced so the saturated first-half DVE
                # (gate 16 + up 16 dequants in ~35 us) sheds work: gate
                # j%4==1 -> ACT+DVE (early kds only; ACT drains before the
                # silus), up j%4==3 -> ACT+GPSIMD (latency-tolerant: all up
                # chains only need to finish by pass 2's start), up j%4==1
                # and down odd -> ACT+DVE, rest direct-DVE off the psum.
                sc = sc_tiles.pop(j)
                meng = nc.vector
                split = (j % 4 == 1) if j < 16 else (j % 2 == 1)
                if split:
                    scb = scbp.tile([P, 2, 512], bf16, name="scb", tag="scb")
                    nc.scalar.copy(scb, sc)
                    sc = scb
                    # gpsimd only for the two up chains consumed last (u11,
                    # u15) — its 2.1 us multiply needs the deadline slack
                    if j in (27, 31):
                        meng = nc.gpsimd
                if j < 16:
                    wr = gwr.tile([P, 2, 512], bf16, name="gwr", tag=f"g{j}")
                    meng.tensor_mul(out=wr, in0=wt_tiles.pop(j), in1=sc)
                    wr_of["g", j] = wr
                elif j < 32:
                    wr = uwr.tile([P, 2, 512], bf16, name="uwr", tag=f"u{j-16}")
                    meng.tensor_mul(out=wr, in0=wt_tiles.pop(j), in1=sc)
                    wr_of["u", j - 16] = wr
                else:
                    m = j - 32
                    h, kf = m // 8, m % 8
                    if h == 0:
                        dwr_tiles[kf] = dwr.tile([P, 4, 512], bf16, name="dwr",
                                                 tag=f"d{kf}")
                    wr = dwr_tiles[kf]
                    wt = wt_tiles[32 + kf] if h == 0 else wt_tiles.pop(32 + kf)
                    meng.tensor_mul(out=wr[:, 2 * h:2 * h + 2],
                                    in0=wt[:, 2 * h:2 * h + 2], in1=sc)
                    wr_of["d", kf] = wr

            def _wt_for_dq(j):
                return j if j < 32 else 32 + (j - 32) % 8

            def ensure_dq(upto):
                while dq_next[0] <= min(upto, 47):
                    jj = dq_next[0]
                    ensure_wt(_wt_for_dq(jj) + 4)
                    if jj % 2 == 0:
                        emit_burst(jj // 2)
                    emit_dq(jj)
                    dq_next[0] += 1

            # prime the pipeline. All rings share 16 DMA engines, so only the
            # two startup-critical transfers (boot -> burst0, wt-g0 -> first
            # dequant) lead, one per ring; everything else staggers behind.
            # x3..x7 ride the otherwise-idle gpsimd SWDGE ring.
            emit_wt(0, nc.scalar)
            load_x_chunk(0, nc.sync)
            load_const("gB4", gB4, nc.scalar)
            load_const("gAT4", gAT4, nc.sync)
            emit_wt(1, nc.scalar)
            wt_next[0] = 2
            load_x_chunk(1, nc.sync)
            load_x_chunk(2, nc.scalar)
            ensure_wt(3)
            warmup(2)
            ensure_dq(1)
            warmup(6)

            h_sb = hpool.tile([P, KF, S], bf16)

            # ---- pass 0: gate fg0, kd-pair-major (dequant-feed limited;
            # psum banks switch every 2 matmuls instead of every 1) ----
            acc = {fi: psacc.tile([P, S], f32, name=f"acc{fi}", tag=f"acc{fi}")
                   for fi in range(4)}
            for kdp in range(KD // 2):
                ensure_dq(min(2 * kdp + 3, 15))
                for fi in range(4):
                    for kd in (2 * kdp, 2 * kdp + 1):
                        nc.tensor.matmul(
                            acc[fi], wr_of["g", kd][:, 0, fi * P:(fi + 1) * P],
                            xs(kd), start=(kd == 0), stop=(kd == KD - 1))
            for fi in range(4):
                nc.scalar.activation(h_sb[:, fi], acc[fi], silu)

            # ---- passes 1-3: gate fg1 / up fg0 / up fg1 ----
            # Full k-contiguous runs: 16 matmuls into ONE psum bank (the
            # b2b rate is 216 ns same-bank vs 259 ns cycling), epilogue of
            # acc[fi] drains behind acc[fi+1]'s run.
            def gu_pass(mat, fg, dq_for_fi, epi):
                a = {fi: psacc.tile([P, S], f32, name=f"acc{fi}",
                                    tag=f"acc{fi}") for fi in range(4)}
                for fi in range(4):
                    for kd in range(KD):
                        if kd == 4:
                            # mid-run so a chain-starved burst has ~2.5 us of
                            # queued matmuls ahead of it instead of stalling
                            # the next run's start
                            ensure_dq(dq_for_fi(fi))
                        nc.tensor.matmul(
                            a[fi],
                            wr_of[mat, kd][:, fg, fi * P:(fi + 1) * P],
                            xs(kd), start=(kd == 0), stop=(kd == KD - 1))
                    epi(fi, a[fi])

            gu_pass("g", 1, lambda fi: 16 + 4 * fi + 3,
                    lambda fi, ac: nc.scalar.activation(h_sb[:, 4 + fi], ac, silu))
            gu_pass("u", 0, lambda fi: 32 + 2 * fi + 1,
                    lambda fi, ac: nc.vector.tensor_mul(
                        out=h_sb[:, fi], in0=h_sb[:, fi], in1=ac))
            gu_pass("u", 1, lambda fi: 40 + 2 * fi + 1,
                    lambda fi, ac: nc.vector.tensor_mul(
                        out=h_sb[:, 4 + fi], in0=h_sb[:, 4 + fi], in1=ac))
            ensure_dq(47)

            # ---- passes 4-7: down mg, k-contiguous runs over kf ----
            for mg in range(4):
                a = {fi: psacc.tile([P, S], f32, name=f"acc{fi}",
                                    tag=f"acc{fi}") for fi in range(4)}
                for fi in range(4):
                    last = (mg == 3 and fi == 3)
                    if not last:
                        for kf in range(KF):
                            nc.tensor.matmul(
                                a[fi],
                                dwr_tiles[kf][:, mg, fi * P:(fi + 1) * P],
                                h_sb[:, kf],
                                start=(kf == 0), stop=(kf == KF - 1))
                        ot = opool.tile([P, S], bf16, name="ot", tag="ot")
                        if fi % 2 == 0:
                            nc.scalar.copy(ot, a[fi])
                        else:
                            nc.vector.tensor_copy(out=ot, in_=a[fi])
                        weng = nc.sync if fi % 2 == 0 else nc.scalar
                        weng.dma_start(
                            out[(mg * 4 + fi) * P:(mg * 4 + fi + 1) * P, :], ot)
                    else:
                        # kernel tail: finish the last tile in s-halves (in
                        # two different psum banks, so half 1's matmuls don't
                        # serialize behind half 0's drain) — the final
                        # copy+store chain after the last matmul is half as
                        # deep, and the exit barrier waits on this store.
                        ph = [a[fi], psacc.tile([P, S], f32, name="acc0",
                                                tag="acc0")]
                        for sh in range(2):
                            hs = slice(sh * 256, (sh + 1) * 256)
                            for kf in range(KF):
                                nc.tensor.matmul(
                                    ph[sh][:, hs],
                                    dwr_tiles[kf][:, mg, fi * P:(fi + 1) * P],
                                    h_sb[:, kf, hs],
                                    start=(kf == 0), stop=(kf == KF - 1))
                            ot = opool.tile([P, 256], bf16, name="otl",
                                            tag="otl")
                            if sh == 0:
                                nc.scalar.copy(ot, ph[sh][:, hs])
                            else:
                                nc.vector.tensor_copy(out=ot, in_=ph[sh][:, hs])
                            weng = nc.sync if sh == 0 else nc.scalar
                            weng.dma_start(
                                out[(mg * 4 + fi) * P:(mg * 4 + fi + 1) * P,
                                    hs], ot)
    nc.finalize()
    return nc


def _prep_inputs(x, gate_snapped, gate_scale_A, gate_scale_B,
                 up_snapped, up_scale_A, up_scale_B,
                 down_snapped, down_scale_A, down_scale_B):
    bf = ml_dtypes.bfloat16
    f = lambda a: np.asarray(a, dtype=np.float32)
    x2 = np.ascontiguousarray(f(x).reshape(D, S).astype(bf))
    gT_full = f(gate_snapped).T      # [D, FF] view
    uT_full = f(up_snapped).T
    dT_full = f(down_snapped).T      # [FF, D] view

    def pack_B4(Bmat, nk):
        # [R, nk*128] fp32 -> [128, nk/2, 128]: strips (0,1)=chunk 2j,
        # strips (2,3)=chunk 2j+1
        b = f(Bmat).reshape(R, nk // 2, 2, P).astype(bf)
        o = np.empty((4 * R, nk // 2, P), dtype=bf)
        o[0 * R:1 * R] = b[:, :, 0, :]
        o[1 * R:2 * R] = b[:, :, 0, :]
        o[2 * R:3 * R] = b[:, :, 1, :]
        o[3 * R:4 * R] = b[:, :, 1, :]
        return o

    def pack_AT4(Amat):
        # A [w, R] -> A^T [R, w] replicated on four strips -> [128, w]
        at = f(Amat).T.astype(bf)
        return np.ascontiguousarray(np.concatenate([at] * 4, axis=0))

    gB_f, uB_f, dB_f = f(gate_scale_B), f(up_scale_B), f(down_scale_B)
    gA_f, uA_f = f(gate_scale_A), f(up_scale_A)
    dAT4 = pack_AT4(down_scale_A)      # [128, D], same for all cores

    in_maps = []
    gB4a = pack_B4(gB_f, KD)
    uB4a = pack_B4(uB_f, KD)
    for c in range(NCORES):
        lo, hi = c * F, (c + 1) * F
        gAT4a = pack_AT4(gA_f[lo:hi])
        in_maps.append({
            "x": x2,
            "gT": np.ascontiguousarray(gT_full[:, lo:hi]).astype(bf)
                    .reshape(KD, P, 2, 512),
            "uT": np.ascontiguousarray(uT_full[:, lo:hi]).astype(bf)
                    .reshape(KD, P, 2, 512),
            "dT": np.ascontiguousarray(dT_full[lo:hi, :]).astype(bf)
                    .reshape(KF, P, 4, 512),
            "gB4": gB4a,
            "uB4": uB4a,
            "dB4": pack_B4(dB_f[:, lo:hi], KF),
            "gAT4": gAT4a,
            "uAT4": pack_AT4(uA_f[lo:hi]),
            "dAT4": dAT4,
            "boot": np.ascontiguousarray(
                np.concatenate([gB4a[:, 0, :], gAT4a], axis=1)),
        })
    return in_maps


def run(trace=False, **inputs):
    if "nc" not in _CACHE:
        _CACHE["nc"] = _build()
    nc = _CACHE["nc"]
    in_maps = _prep_inputs(**inputs)
    try:
        res = run_bass_kernel_spmd(nc, in_maps, list(range(NCORES)), trace=trace)
    except Exception:
        # A transient device flake (NRT_EXEC_UNIT_UNRECOVERABLE) poisons the
        # PJRT client for the process; tearing the backend down and
        # reconnecting recovers it the same way a fresh process does.
        try:
            import jax.extend.backend
            jax.extend.backend.clear_backends()
        except Exception:
            pass
        res = run_bass_kernel_spmd(nc, in_maps, list(range(NCORES)), trace=trace)
    partial = np.zeros((D, S), dtype=np.float32)
    for c in range(NCORES):
        partial += np.asarray(res.results[c]["out"], dtype=np.float32)
    return partial.reshape(1, D, 1, S), res


def kernel(**inputs):
    out, _ = run(trace=False, **inputs)
    return out


if __name__ == "__main__":
    rng = np.random.default_rng(0)
    ins = {
        "x": rng.standard_normal((1, D, 1, S)).astype(np.float32),
        "gate_snapped": (rng.standard_normal((FF, D)) * 0.02).astype(np.float32),
        "gate_scale_A": (rng.standard_normal((FF, R)) * 0.1).astype(np.float32),
        "gate_scale_B": (rng.standard_normal((R, D)) * 0.1).astype(np.float32),
        "up_snapped": (rng.standard_normal((FF, D)) * 0.02).astype(np.float32),
        "up_scale_A": (rng.standard_normal((FF, R)) * 0.1).astype(np.float32),
        "up_scale_B": (rng.standard_normal((R, D)) * 0.1).astype(np.float32),
        "down_snapped": (rng.standard_normal((D, FF)) * 0.02).astype(np.float32),
        "down_scale_A": (rng.standard_normal((D, R)) * 0.1).astype(np.float32),
        "down_scale_B": (rng.standard_normal((R, FF)) * 0.1).astype(np.float32),
    }
    out = kernel(**ins)
    print("kernel ran, out shape", out.shape, "mean abs", np.abs(out).mean())


# revision 42
# speedup vs baseline: 1.0674x; 1.0674x over previous
"""Trainium2 Bass kernel for FFNWithScales (SwiGLU MLP with low-rank dequant scales).

Reference computation (all fp32):
    gate_eff = gate_snapped * (gate_scale_A @ gate_scale_B)       # [8192, 2048]
    up_eff   = up_snapped   * (up_scale_A   @ up_scale_B)         # [8192, 2048]
    down_eff = down_snapped * (down_scale_A @ down_scale_B)       # [2048, 8192]
    h   = silu(gate_eff @ x) * (up_eff @ x)                       # [8192, 512]
    out = down_eff @ h                                            # [2048, 512]

Sharding (8 cores, tensor-parallel on d_ff): core c owns d_ff rows
[c*1024, (c+1)*1024) of gate/up (and the matching columns of down).
Each core computes a full-[2048, 512] partial of the down projection;
partials are summed on the host (the all-reduce step).

Kernel design (v2 — PE-bound, so everything serves the PE stream):
  - All tensors ship bf16 from the host (snapped weights included): the
    extra bf16 rounding of snapped costs ~1e-3 relative error against a
    2e-2 budget, and it halves HBM traffic so DMA (~17 MB @ ~300 GB/s)
    stays far under the PE streaming time.
  - Weights are pre-transposed on host so their contraction dim rides the
    partitions, and are DMA'd in full-row tiles (2-4 KB contiguous per
    partition line): gate/up as [128 d, 1024 f] per d-chunk, down as
    [128 f, 2048 d] per f-chunk.
  - The rank-32 scale products run 4-way row-packed on the PE
    (tile_position strips 0/32/64/96): one ~280 ns stream covers two
    d-chunks' worth of scale tiles. The DVE dequant-multiplies the bf16
    snapped tile by the fp32 psum scale tile, emitting the bf16 wr tile
    the main matmuls consume. Dequanted gate/up/down weights stay
    resident in SBUF so each of the 48 dequants serves two passes.
  - Main matmuls are kd-major in the first pass (matches the dequant
    feed rate) and fi-major-blocked afterwards, so a psum accumulator's
    epilogue (silu / up-multiply / output copy) always drains behind
    12+ matmuls on other banks — pass boundaries never stall the PE.
  - Six dummy warm-up matmuls on a memset tile run while the first DMAs
    land, so the PE HAM clock-gate reaches 2.4 GHz before real work.
  - Output partials store bf16 (host accumulates in fp32), with the
    final pass's stores split across both HWDGE rings for a short tail.
"""

import numpy as np
import ml_dtypes

import concourse.bass as bass
from concourse import bacc
import concourse.mybir as mybir
from concourse.tile import TileContext
from concourse.bass_utils import run_bass_kernel_spmd

P = 128
D = 2048        # d_model
FF = 8192       # d_ff (global)
S = 512         # sequence
R = 32          # rank
NCORES = 8
F = FF // NCORES          # 1024 local d_ff rows
KD = D // P               # 16 d_model chunks
KF = F // P               # 8 local d_ff chunks

f32 = mybir.dt.float32
bf16 = mybir.dt.bfloat16

_CACHE = {}


def _build():
    nc = bacc.Bacc()
    x = nc.declare_dram_parameter("x", [D, S], bf16, isOutput=False)
    # snapped weights, transposed, tiled so a dram slice is an SBUF tile
    gT = nc.declare_dram_parameter("gT", [KD, P, 2, 512], bf16, isOutput=False)
    uT = nc.declare_dram_parameter("uT", [KD, P, 2, 512], bf16, isOutput=False)
    dT = nc.declare_dram_parameter("dT", [KF, P, 4, 512], bf16, isOutput=False)
    # 4-way packed scale factors: B strips for chunk pairs (both fg copies),
    # A^T replicated on all four 32-row strips.
    gB4 = nc.declare_dram_parameter("gB4", [4 * R, KD // 2, P], bf16, isOutput=False)
    uB4 = nc.declare_dram_parameter("uB4", [4 * R, KD // 2, P], bf16, isOutput=False)
    dB4 = nc.declare_dram_parameter("dB4", [4 * R, KF // 2, P], bf16, isOutput=False)
    gAT4 = nc.declare_dram_parameter("gAT4", [4 * R, F], bf16, isOutput=False)
    uAT4 = nc.declare_dram_parameter("uAT4", [4 * R, F], bf16, isOutput=False)
    dAT4 = nc.declare_dram_parameter("dAT4", [4 * R, D], bf16, isOutput=False)
    # boot = [gB4 pair 0 | gAT4] in one transfer so scale-burst 0 can fire
    # ~1.7 us earlier than waiting for both full factor loads
    boot = nc.declare_dram_parameter("boot", [4 * R, P + F], bf16, isOutput=False)
    out = nc.declare_dram_parameter("out", [D, S], bf16, isOutput=True)

    silu = mybir.ActivationFunctionType.Silu

    with TileContext(nc) as tc:
        with (
            tc.tile_pool(name="const", bufs=1) as const,
            tc.tile_pool(name="wtg", bufs=6) as wtg,
            tc.tile_pool(name="wtd", bufs=8) as wtd,
            tc.tile_pool(name="gwr", bufs=1) as gwr,
            tc.tile_pool(name="uwr", bufs=1) as uwr,
            tc.tile_pool(name="dwr", bufs=1) as dwr,
            tc.tile_pool(name="hbuf", bufs=1) as hpool,
            tc.tile_pool(name="scb", bufs=3) as scbp,
            tc.tile_pool(name="obuf", bufs=3) as opool,
            tc.tile_pool(name="psacc", bufs=1, space="PSUM") as psacc,
            tc.tile_pool(name="pssc", bufs=2, space="PSUM") as pssc,
        ):
            # ---- constant loads (factors lead the rings, x0/x1 next) ----
            rounded = {}

            def load_const(nm, dram, eng):
                rt = const.tile(list(dram.shape), bf16, name=nm, tag=nm)
                eng.dma_start(rt, dram[:])
                rounded[nm] = rt

            load_const("boot", boot, nc.sync)

            x_sb = [None] * (KD // 2)

            def load_x_chunk(q, eng):
                xt = const.tile([P, 2, S], bf16, name=f"x{q}", tag=f"x{q}")
                eng.dma_start(
                    xt, x[q * 2 * P:(q + 1) * 2 * P, :].rearrange(
                        "(ko p) s -> p ko s", p=P))
                x_sb[q] = xt

            def xs(kd):
                return x_sb[kd // 2][:, kd % 2]

            # ---- PE warm-up: dummy matmuls cycling the ACC banks (NOT the
            # sc slots, which burst 0 needs as soon as `boot` lands).
            # Emitted in two chunks around the first dequant chain, because
            # the PE stream order is fixed at compile time: burst 0 must sit
            # only ~2 warmups deep so the chain starts the moment boot lands.
            junk = const.tile([P, 640], bf16, name="junk", tag="junk")
            nc.vector.memset(junk, 0.0)

            wu_ctr = [0]

            def warmup(n):
                for _ in range(n):
                    i = wu_ctr[0] % 4
                    wu_ctr[0] += 1
                    wps = psacc.tile([P, S], f32, name=f"acc{i}", tag=f"acc{i}")
                    nc.tensor.matmul(wps, junk[:, 0:128], junk[:, 128:640],
                                     start=True, stop=True)

            # ---- weight stream: 40 snapped-tile DMAs ----
            # jobs 0..15 gate kd, 16..31 up kd, 32..39 down kf
            wt_tiles = {}
            dma_parity = [0]

            def ring():
                dma_parity[0] ^= 1
                return nc.sync if dma_parity[0] else nc.scalar

            def emit_wt(j, eng=None):
                if j < 16:
                    t = wtg.tile([P, 2, 512], bf16, name="wt", tag="wt")
                    (eng or ring()).dma_start(t, gT[j])
                elif j < 32:
                    t = wtg.tile([P, 2, 512], bf16, name="wt", tag="wt")
                    (eng or ring()).dma_start(t, uT[j - 16])
                else:
                    t = wtd.tile([P, 4, 512], bf16, name="wtd", tag="wtd")
                    (eng or ring()).dma_start(t, dT[j - 32])
                wt_tiles[j] = t
                if j == 7:
                    # up/down factors ride behind the first weight tiles
                    # (first needed ~30 us in, land ~18)
                    for nm, dram in (("uB4", uB4), ("uAT4", uAT4),
                                     ("dB4", dB4), ("dAT4", dAT4)):
                        load_const(nm, dram, ring())

            # ---- scale bursts + copies + dequants ----
            # dq jobs: 0..15 gate kd, 16..31 up kd, 32..47 down (h*8 + kf)
            # burst b covers dq jobs 2b, 2b+1. Each job is a 3-engine chain:
            # PE scale-matmul -> ACT psum->sbuf bf16 copy -> DVE bf16 2x
            # dequant multiply (all-16-bit keeps the DVE in its fast mode).
            sc_tiles = {}
            scb_tiles = {}
            dwr_tiles = {}
            dq_next = [0]
            wt_next = [0]

            def ensure_wt(upto):
                while wt_next[0] <= min(upto, 39):
                    emit_wt(wt_next[0])
                    wt_next[0] += 1

            def emit_burst(b):
                sc_a = pssc.tile([P, 2, S], f32, name="sc", tag="sc")
                sc_b = pssc.tile([P, 2, S], f32, name="sc", tag="sc")
                cols = [0, 512, 0, 512]
                if b == 0:         # gate pair 0 from the boot concat
                    bt = rounded["boot"]
                    Bsl = lambda i: bt[i * R:(i + 1) * R, 0:P]
                    Asl = lambda i, c: bt[i * R:(i + 1) * R, P + c:P + c + 512]
                elif b < 8:        # gate kd pair (2b, 2b+1)
                    Bm, Am = rounded["gB4"], rounded["gAT4"]
                    Bsl = lambda i: Bm[i * R:(i + 1) * R, b]
                    Asl = lambda i, c: Am[i * R:(i + 1) * R, c:c + 512]
                elif b < 16:       # up kd pair
                    Bm, Am = rounded["uB4"], rounded["uAT4"]
                    Bsl = lambda i: Bm[i * R:(i + 1) * R, b - 8]
                    Asl = lambda i, c: Am[i * R:(i + 1) * R, c:c + 512]
                else:              # down: m = b-16: h = m//4, kf pair j = m%4
                    m = b - 16
                    h = m // 4
                    Bm, Am, jd = rounded["dB4"], rounded["dAT4"], m % 4
                    Bsl = lambda i: Bm[i * R:(i + 1) * R, jd]
                    Asl = lambda i, c: Am[i * R:(i + 1) * R, c:c + 512]
                    cols = [h * 1024, h * 1024 + 512,
                            h * 1024, h * 1024 + 512]
                for i, dst in enumerate((sc_a[:, 0], sc_a[:, 1],
                                         sc_b[:, 0], sc_b[:, 1])):
                    nc.tensor.matmul(
                        dst, Bsl(i), Asl(i, cols[i]),
                        start=True, stop=True,
                        tile_position=(R * i, 0),
                    )
                sc_tiles[2 * b] = sc_a
                sc_tiles[2 * b + 1] = sc_b

            wr_of = {}

            def emit_dq(j):
                # three chain flavours, balanced across DVE/ACT/GPSIMD so no
                # single engine's backlog can stall the PE's sc-slot ring:
                #   j%4 in (0,2): DVE multiplies straight off the sc psum
                #   j%4 == 1:     ACT copies psum->sbuf, DVE multiplies (2x)
                #   j%4 == 3:     ACT copies psum->sbuf, GPSIMD multiplies
                # Chain flavours: gate (j<16) feeds pass 0 just-in-time AND
                # its pass ends right before the silus, so gate chains stay
                # entirely on the DVE (stable latency, ACT queue empty for
                # the silus). Up/down chains are prefetched with slack, so
                # they spread across ACT-copy + DVE/GPSIMD-multiply paths.
                # Chain flavours, balanced so the saturated first-half DVE
                # (gate 16 + up 16 dequants in ~35 us) sheds work: gate
                # j%4==1 -> ACT+DVE (early kds only; ACT drains before the
                # silus), up j%4==3 -> ACT+GPSIMD (latency-tolerant: all up
                # chains only need to finish by pass 2's start), up j%4==1
                # and down odd -> ACT+DVE, rest direct-DVE off the psum.
                sc = sc_tiles.pop(j)
                meng = nc.vector
                split = (j % 4 == 1) if j < 16 else (j % 2 == 1)
                if split:
                    scb = scbp.tile([P, 2, 512], bf16, name="scb", tag="scb")
                    nc.scalar.copy(scb, sc)
                    sc = scb
                    if 16 <= j < 32 and j % 4 == 3:
                        meng = nc.gpsimd
                if j < 16:
                    wr = gwr.tile([P, 2, 512], bf16, name="gwr", tag=f"g{j}")
                    meng.tensor_mul(out=wr, in0=wt_tiles.pop(j), in1=sc)
                    wr_of["g", j] = wr
                elif j < 32:
                    wr = uwr.tile([P, 2, 512], bf16, name="uwr", tag=f"u{j-16}")
                    meng.tensor_mul(out=wr, in0=wt_tiles.pop(j), in1=sc)
                    wr_of["u", j - 16] = wr
                else:
                    m = j - 32
                    h, kf = m // 8, m % 8
                    if h == 0:
                        dwr_tiles[kf] = dwr.tile([P, 4, 512], bf16, name="dwr",
                                                 tag=f"d{kf}")
                    wr = dwr_tiles[kf]
                    wt = wt_tiles[32 + kf] if h == 0 else wt_tiles.pop(32 + kf)
                    meng.tensor_mul(out=wr[:, 2 * h:2 * h + 2],
                                    in0=wt[:, 2 * h:2 * h + 2], in1=sc)
                    wr_of["d", kf] = wr

            def _wt_for_dq(j):
                return j if j < 32 else 32 + (j - 32) % 8

            def ensure_dq(upto):
                while dq_next[0] <= min(upto, 47):
                    jj = dq_next[0]
                    ensure_wt(_wt_for_dq(jj) + 4)
                    if jj % 2 == 0:
                        emit_burst(jj // 2)
                    emit_dq(jj)
                    dq_next[0] += 1

            # prime the pipeline. All rings share 16 DMA engines, so the
            # startup-critical transfers (boot, wt-g0, x0 — the burst0 ->
            # dequant -> first-matmul chain) lead both rings; the bulk
            # factor loads queue behind them.
            emit_wt(0, nc.sync)
            load_x_chunk(0, nc.scalar)
            load_const("gB4", gB4, nc.scalar)
            emit_wt(1, nc.sync)
            load_const("gAT4", gAT4, nc.scalar)
            wt_next[0] = 2
            load_x_chunk(1, nc.scalar)
            ensure_wt(3)
            warmup(2)
            ensure_dq(1)
            warmup(6)

            h_sb = hpool.tile([P, KF, S], bf16)

            # ---- pass 0: gate fg0, kd-pair-major (dequant-feed limited;
            # psum banks switch every 2 matmuls instead of every 1) ----
            acc = {fi: psacc.tile([P, S], f32, name=f"acc{fi}", tag=f"acc{fi}")
                   for fi in range(4)}
            load_x_chunk(2, nc.scalar)
            for kdp in range(KD // 2):
                ensure_dq(min(2 * kdp + 3, 15))
                if kdp <= 4:
                    load_x_chunk(kdp + 3, ring())
                for fi in range(4):
                    for kd in (2 * kdp, 2 * kdp + 1):
                        nc.tensor.matmul(
                            acc[fi], wr_of["g", kd][:, 0, fi * P:(fi + 1) * P],
                            xs(kd), start=(kd == 0), stop=(kd == KD - 1))
            for fi in range(4):
                nc.scalar.activation(h_sb[:, fi], acc[fi], silu)

            # ---- passes 1-3: gate fg1 / up fg0 / up fg1 ----
            # Full k-contiguous runs: 16 matmuls into ONE psum bank (the
            # b2b rate is 216 ns same-bank vs 259 ns cycling), epilogue of
            # acc[fi] drains behind acc[fi+1]'s run.
            def gu_pass(mat, fg, dq_for_fi, epi):
                a = {fi: psacc.tile([P, S], f32, name=f"acc{fi}",
                                    tag=f"acc{fi}") for fi in range(4)}
                for fi in range(4):
                    for kd in range(KD):
                        if kd == 4:
                            # mid-run so a chain-starved burst has ~2.5 us of
                            # queued matmuls ahead of it instead of stalling
                            # the next run's start
                            ensure_dq(dq_for_fi(fi))
                        nc.tensor.matmul(
                            a[fi],
                            wr_of[mat, kd][:, fg, fi * P:(fi + 1) * P],
                            xs(kd), start=(kd == 0), stop=(kd == KD - 1))
                    epi(fi, a[fi])

            gu_pass("g", 1, lambda fi: 16 + 4 * fi + 3,
                    lambda fi, ac: nc.scalar.activation(h_sb[:, 4 + fi], ac, silu))
            gu_pass("u", 0, lambda fi: 32 + 2 * fi + 1,
                    lambda fi, ac: nc.vector.tensor_mul(
                        out=h_sb[:, fi], in0=h_sb[:, fi], in1=ac))
            gu_pass("u", 1, lambda fi: 40 + 2 * fi + 1,
                    lambda fi, ac: nc.vector.tensor_mul(
                        out=h_sb[:, 4 + fi], in0=h_sb[:, 4 + fi], in1=ac))
            ensure_dq(47)

            # ---- passes 4-7: down mg, k-contiguous runs over kf ----
            for mg in range(4):
                a = {fi: psacc.tile([P, S], f32, name=f"acc{fi}",
                                    tag=f"acc{fi}") for fi in range(4)}
                for fi in range(4):
                    last = (mg == 3 and fi == 3)
                    if not last:
                        for kf in range(KF):
                            nc.tensor.matmul(
                                a[fi],
                                dwr_tiles[kf][:, mg, fi * P:(fi + 1) * P],
                                h_sb[:, kf],
                                start=(kf == 0), stop=(kf == KF - 1))
                        ot = opool.tile([P, S], bf16, name="ot", tag="ot")
                        if fi % 2 == 0:
                            nc.scalar.copy(ot, a[fi])
                        else:
                            nc.vector.tensor_copy(out=ot, in_=a[fi])
                        weng = nc.sync if fi % 2 == 0 else nc.scalar
                        weng.dma_start(
                            out[(mg * 4 + fi) * P:(mg * 4 + fi + 1) * P, :], ot)
                    else:
                        # kernel tail: finish the last tile in s-halves (in
                        # two different psum banks, so half 1's matmuls don't
                        # serialize behind half 0's drain) — the final
                        # copy+store chain after the last matmul is half as
                        # deep, and the exit barrier waits on this store.
                        ph = [a[fi], psacc.tile([P, S], f32, name="acc0",
                                                tag="acc0")]
                        for sh in range(2):
                            hs = slice(sh * 256, (sh + 1) * 256)
                            for kf in range(KF):
                                nc.tensor.matmul(
                                    ph[sh][:, hs],
                                    dwr_tiles[kf][:, mg, fi * P:(fi + 1) * P],
                                    h_sb[:, kf, hs],
                                    start=(kf == 0), stop=(kf == KF - 1))
                            ot = opool.tile([P, 256], bf16, name="otl",
                                            tag="otl")
                            if sh == 0:
                                nc.scalar.copy(ot, ph[sh][:, hs])
                            else:
                                nc.vector.tensor_copy(out=ot, in_=ph[sh][:, hs])
                            weng = nc.sync if sh == 0 else nc.scalar
                            weng.dma_start(
                                out[(mg * 4 + fi) * P:(mg * 4 + fi + 1) * P,
                                    hs], ot)
    nc.finalize()
    return nc


def _prep_inputs(x, gate_snapped, gate_scale_A, gate_scale_B,
                 up_snapped, up_scale_A, up_scale_B,
                 down_snapped, down_scale_A, down_scale_B):
    bf = ml_dtypes.bfloat16
    f = lambda a: np.asarray(a, dtype=np.float32)
    x2 = np.ascontiguousarray(f(x).reshape(D, S).astype(bf))
    gT_full = f(gate_snapped).T      # [D, FF] view
    uT_full = f(up_snapped).T
    dT_full = f(down_snapped).T      # [FF, D] view

    def pack_B4(Bmat, nk):
        # [R, nk*128] fp32 -> [128, nk/2, 128]: strips (0,1)=chunk 2j,
        # strips (2,3)=chunk 2j+1
        b = f(Bmat).reshape(R, nk // 2, 2, P).astype(bf)
        o = np.empty((4 * R, nk // 2, P), dtype=bf)
        o[0 * R:1 * R] = b[:, :, 0, :]
        o[1 * R:2 * R] = b[:, :, 0, :]
        o[2 * R:3 * R] = b[:, :, 1, :]
        o[3 * R:4 * R] = b[:, :, 1, :]
        return o

    def pack_AT4(Amat):
        # A [w, R] -> A^T [R, w] replicated on four strips -> [128, w]
        at = f(Amat).T.astype(bf)
        return np.ascontiguousarray(np.concatenate([at] * 4, axis=0))

    gB_f, uB_f, dB_f = f(gate_scale_B), f(up_scale_B), f(down_scale_B)
    gA_f, uA_f = f(gate_scale_A), f(up_scale_A)
    dAT4 = pack_AT4(down_scale_A)      # [128, D], same for all cores

    in_maps = []
    gB4a = pack_B4(gB_f, KD)
    uB4a = pack_B4(uB_f, KD)
    for c in range(NCORES):
        lo, hi = c * F, (c + 1) * F
        gAT4a = pack_AT4(gA_f[lo:hi])
        in_maps.append({
            "x": x2,
            "gT": np.ascontiguousarray(gT_full[:, lo:hi]).astype(bf)
                    .reshape(KD, P, 2, 512),
            "uT": np.ascontiguousarray(uT_full[:, lo:hi]).astype(bf)
                    .reshape(KD, P, 2, 512),
            "dT": np.ascontiguousarray(dT_full[lo:hi, :]).astype(bf)
                    .reshape(KF, P, 4, 512),
            "gB4": gB4a,
            "uB4": uB4a,
            "dB4": pack_B4(dB_f[:, lo:hi], KF),
            "gAT4": gAT4a,
            "uAT4": pack_AT4(uA_f[lo:hi]),
            "dAT4": dAT4,
            "boot": np.ascontiguousarray(
                np.concatenate([gB4a[:, 0, :], gAT4a], axis=1)),
        })
    return in_maps


def run(trace=False, **inputs):
    if "nc" not in _CACHE:
        _CACHE["nc"] = _build()
    nc = _CACHE["nc"]
    in_maps = _prep_inputs(**inputs)
    try:
        res = run_bass_kernel_spmd(nc, in_maps, list(range(NCORES)), trace=trace)
    except Exception:
        # A transient device flake (NRT_EXEC_UNIT_UNRECOVERABLE) poisons the
        # PJRT client for the process; tearing the backend down and
        # reconnecting recovers it the same way a fresh process does.
        try:
            import jax.extend.backend
            jax.extend.backend.clear_backends()
        except Exception:
            pass
        res = run_bass_kernel_spmd(nc, in_maps, list(range(NCORES)), trace=trace)
    partial = np.zeros((D, S), dtype=np.float32)
    for c in range(NCORES):
        partial += np.asarray(res.results[c]["out"], dtype=np.float32)
    return partial.reshape(1, D, 1, S), res


def kernel(**inputs):
    out, _ = run(trace=False, **inputs)
    return out


if __name__ == "__main__":
    rng = np.random.default_rng(0)
    ins = {
        "x": rng.standard_normal((1, D, 1, S)).astype(np.float32),
        "gate_snapped": (rng.standard_normal((FF, D)) * 0.02).astype(np.float32),
        "gate_scale_A": (rng.standard_normal((FF, R)) * 0.1).astype(np.float32),
        "gate_scale_B": (rng.standard_normal((R, D)) * 0.1).astype(np.float32),
        "up_snapped": (rng.standard_normal((FF, D)) * 0.02).astype(np.float32),
        "up_scale_A": (rng.standard_normal((FF, R)) * 0.1).astype(np.float32),
        "up_scale_B": (rng.standard_normal((R, D)) * 0.1).astype(np.float32),
        "down_snapped": (rng.standard_normal((D, FF)) * 0.02).astype(np.float32),
        "down_scale_A": (rng.standard_normal((D, R)) * 0.1).astype(np.float32),
        "down_scale_B": (rng.standard_normal((R, FF)) * 0.1).astype(np.float32),
    }
    out = kernel(**ins)
    print("kernel ran, out shape", out.shape, "mean abs", np.abs(out).mean())
